# revision 29
# baseline (speedup 1.0000x reference)
"""GAT regression kernel for Trainium2, distributed over 8 NeuronCores.

Strategy (dst-node sharding + bulk dma_gather edge fetch):
  - Each core owns a contiguous range of destination nodes (N/8). Host sorts
    the 1.2M random edges by dst (self-loops are handled analytically on
    device, never gathered), shards them by dst range, and groups them into
    128-node dst tiles.
  - The per-layer node table (one 256B row per node: [h(64) | 1 | s_src |
    s_dst | pad] bf16) is computed locally and AllGathered, so each core can
    gather any src row.
  - dma_gather has int16 indices, so the 100352-row table is addressed
    through NR=5 overlapping 32768-row windows; each tile's edges are
    balance-assigned to windows (chain-greedy) so every (tile, window) fits
    in CFIX=3 128-edge chunks. One dma_gather per (8-tile batch, window)
    fetches ~3072 rows in a single instruction.
  - Per 128-edge chunk: attention scores z = s_src[src] + s_dst[dst] use a
    transposed one-hot matmul to expand the tile's s_dst column per edge;
    w = exp(leaky_relu(z)); a w-scaled one-hot feeds a PE matmul that
    segment-sums w*[h_src | 1] into PSUM per dst tile.
  - Self-loop contribution (w_self*[h_own | 1]) is added in the finalize from
    SBUF-resident per-node h / scores. Layer-1 output is PE-transposed per
    tile and stored as [64, npad] so layer 2's node phase reads it directly.
  - Layer-2 outputs are column-summed via ones-matmuls, AllReduced, and the
    final linear head produces the [1,1] output on every core.
"""

import sys

for _p in ("/opt/trn_rl_repo", "/opt/trn_rl_repo/concourse"):
    if _p not in sys.path:
        sys.path.insert(0, _p)

import numpy as np
import ml_dtypes

import concourse.bass as bass
import concourse.mybir as mybir
import concourse.tile as tile
from concourse import bacc

P = 128
D = 64            # feature dim (both layers)
DT = 128          # table row: [h(0:64), one(64), s_src(65), s_dst(66), pad] (256B)
CORES = 8
NEG = 0.2         # leaky relu slope
NR = 5            # gather index windows (int16 limit: 32768 rows per window)
WIN = 32768
CAPS = (3, 3, 2, 3, 3)   # chunk capacity per (tile, window g)
CPT = sum(CAPS)          # chunks per tile (14)
CFIX = 3                 # legacy name kept for the harness signature
TBN = 8           # node-phase tiles per batch
TBE = 8           # edge-phase tiles per batch

N_NODES = 100000
E_RAW = 1200000

f32 = mybir.dt.float32
bf16 = mybir.dt.bfloat16
i32 = mybir.dt.int32
i16 = mybir.dt.int16
bf16_np = ml_dtypes.bfloat16


class Cfg:
    def __init__(self, n_nodes):
        assert n_nodes % CORES == 0
        self.n_nodes = n_nodes
        self.nown = n_nodes // CORES          # real nodes per core
        self.nt = -(-self.nown // P)          # dst tiles per core
        self.npad = self.nt * P               # padded nodes per core
        self.ntab = CORES * self.npad         # padded global table rows
        self.last_rows = self.nown - (self.nt - 1) * P
        self.ctot = CPT * self.nt             # chunk slots per core per layer
        step = (self.ntab - WIN) // (NR - 1)
        self.bases = [g * step for g in range(NR - 1)] + [self.ntab - WIN]


def _pad_gid(node_ids, cfg):
    """global node id -> row in the padded AllGather table"""
    return (node_ids // cfg.nown) * cfg.npad + (node_ids % cfg.nown)


def prep_inputs(inputs, cfg):
    """Host-side sharding: index manipulation + dtype/layout prep only."""
    x = np.asarray(inputs["x"], np.float32)
    ei = np.asarray(inputs["edge_index"], np.int64)
    src = ei[0]
    dst = ei[1]
    order = np.argsort(dst, kind="stable")
    src_s, dst_s = src[order], dst[order]
    rows_s = _pad_gid(src_s, cfg)
    bases = np.asarray(cfg.bases, np.int64)
    caps = np.asarray(CAPS, np.int64) * P

    core_bounds = np.searchsorted(dst_s, np.arange(CORES + 1) * cfg.nown)

    w1 = np.asarray(inputs["W1"], np.float32)
    w2 = np.asarray(inputs["W2"], np.float32)

    def _aug(w, a_s, a_d):
        m = np.zeros((D, DT), np.float32)
        m[:, 0:D] = w
        m[:, D + 1] = w @ np.asarray(a_s, np.float32)
        m[:, D + 2] = w @ np.asarray(a_d, np.float32)
        return m

    shared = {
        "waug1": _aug(w1, inputs["a_src1"], inputs["a_dst1"]),
        "waug2": _aug(w2, inputs["a_src2"], inputs["a_dst2"]),
        "b1t": np.ascontiguousarray(
            np.broadcast_to(np.asarray(inputs["b1"], np.float32), (P, D))),
        "b2t": np.ascontiguousarray(
            np.broadcast_to(np.asarray(inputs["b2"], np.float32), (P, D))),
        "woutt": np.asarray(inputs["W_out"], np.float32).reshape(1, D),
        "bout": np.asarray(inputs["b_out"], np.float32).reshape(1, 1),
    }

    in_maps = []
    for k in range(CORES):
        sl = slice(core_bounds[k], core_bounds[k + 1])
        dloc = dst_s[sl] - k * cfg.nown
        rows = rows_s[sl]
        tb = np.searchsorted(dloc, np.arange(cfg.nt + 1) * P)

        idxA = np.zeros((cfg.ctot, P), np.int16)
        erelA = np.full((cfg.ctot, P), -1.0, np.float32)

        for t in range(cfg.nt):
            e0, e1 = tb[t], tb[t + 1]
            r = rows[e0:e1]
            drel = (dloc[e0:e1] - t * P).astype(np.int32)
            lo = np.searchsorted(bases, r - WIN, side='right')
            hi = np.searchsorted(bases, r, side='right') - 1
            assign = np.where(lo == hi, lo, -1).astype(np.int64)
            loads = np.bincount(lo[lo == hi], minlength=NR)
            # chain greedy: movable edges (two eligible windows) fill left
            for g in range(NR - 1):
                mv = np.flatnonzero((lo == g) & (hi == g + 1))
                kk = max(0, min(len(mv), int(caps[g]) - int(loads[g])))
                assign[mv[:kk]] = g
                assign[mv[kk:]] = g + 1
                loads[g] += kk
                loads[g + 1] += len(mv) - kk
            if (loads > caps).any():
                raise RuntimeError(
                    f"window overflow core {k} tile {t}: {loads}")
            b = t // TBE
            t0 = b * TBE
            nb = min(TBE, cfg.nt - t0)
            base_b = CPT * t0
            goff = 0
            for g in range(NR):
                el = np.flatnonzero(assign == g)
                s0 = base_b + goff * nb + (t - t0) * CAPS[g]
                goff += CAPS[g]
                n = len(el)
                cap = int(caps[g])
                # pads must not all hit one row (HBM bank-conflict
                # serialization) — spread them across the window
                iv = ((np.arange(cap, dtype=np.int64) * 2654435761
                       + t * 977 + g * 131) % WIN).astype(np.int16)
                rv = np.full(cap, -1.0, np.float32)
                iv[:n] = (r[el] - bases[g]).astype(np.int16)
                rv[:n] = drel[el]
                idxA[s0:s0 + CAPS[g]] = iv.reshape(CAPS[g], P)
                erelA[s0:s0 + CAPS[g]] = rv.reshape(CAPS[g], P)

        # dma_gather index layout: idx j of a call at [j%16, j//16];
        # chunk slot c occupies columns [8c, 8c+8). Replicate to 128 rows.
        idx16 = np.transpose(
            idxA.reshape(cfg.ctot, 8, 16), (2, 0, 1)).reshape(16, cfg.ctot * 8)
        idx16 = np.ascontiguousarray(np.tile(idx16, (8, 1)))

        xt = np.zeros((D, cfg.npad), np.float32)
        xt[:, :cfg.nown] = x[k * cfg.nown:(k + 1) * cfg.nown].T
        in_maps.append({
            "xt": xt,
            "idx16": idx16,
            "erel": np.ascontiguousarray(erelA.T),
            "erelrow": erelA.astype(bf16_np),
            **shared,
        })
    return in_maps, CFIX


def build_kernel(cfg, c_fix=CFIX, debug_dumps=False, rep=1, rep_mode="all"):
    """Build the SPMD Bass program (same program for all 8 cores)."""
    nc = bacc.Bacc("TRN2", target_bir_lowering=False, debug=False,
                   num_devices=CORES, dynamic_dma_scratch_size=32768)

    ctot = cfg.ctot
    # I/O
    xt_d = nc.dram_tensor("xt", [D, cfg.npad], f32, kind="ExternalInput")
    idx_d = nc.dram_tensor("idx16", [P, ctot * 8], i16, kind="ExternalInput")
    erel_d = nc.dram_tensor("erel", [P, ctot], f32, kind="ExternalInput")
    erelrow_d = nc.dram_tensor("erelrow", [ctot, P], bf16, kind="ExternalInput")
    waug1_d = nc.dram_tensor("waug1", [D, DT], f32, kind="ExternalInput")
    waug2_d = nc.dram_tensor("waug2", [D, DT], f32, kind="ExternalInput")
    b1t_d = nc.dram_tensor("b1t", [P, D], f32, kind="ExternalInput")
    b2t_d = nc.dram_tensor("b2t", [P, D], f32, kind="ExternalInput")
    woutt_d = nc.dram_tensor("woutt", [1, D], f32, kind="ExternalInput")
    bout_d = nc.dram_tensor("bout", [1, 1], f32, kind="ExternalInput")
    out_d = nc.dram_tensor("out", [1, 1], f32, kind="ExternalOutput")

    # internal DRAM
    h1own = nc.dram_tensor("h1own", [cfg.npad, DT], bf16)
    tab1 = nc.dram_tensor("tab1", [cfg.ntab, DT], bf16, addr_space="Shared")
    out1t = nc.dram_tensor("out1t", [D, cfg.npad], bf16)
    h2own = nc.dram_tensor("h2own", [cfg.npad, DT], bf16)
    tab2 = nc.dram_tensor("tab2", [cfg.ntab, DT], bf16, addr_space="Shared")
    gin = nc.dram_tensor("gin", [1, D], f32)
    gout = nc.dram_tensor("gout", [1, D], f32, addr_space="Shared")

    rg = [list(range(CORES))]
    AF = mybir.ActivationFunctionType
    OP = mybir.AluOpType

    with tile.TileContext(nc) as tc:
        with (
            tc.tile_pool(name="const", bufs=1) as cpool,
            tc.tile_pool(name="sbuf", bufs=2) as sb,
            tc.tile_pool(name="gp", bufs=3) as gp,
            tc.tile_pool(name="oh", bufs=6) as ohp,
            tc.tile_pool(name="psn", bufs=2, space="PSUM") as psn,
            tc.tile_pool(name="pse", bufs=2, space="PSUM") as pse,
            tc.tile_pool(name="psg", bufs=1, space="PSUM") as psg,
            tc.tile_pool(name="pst", bufs=1, space="PSUM") as pst,
            tc.tile_pool(name="psr", bufs=1, space="PSUM") as psr,
            tc.tile_pool(name="psd", bufs=1, space="PSUM") as psd,
        ):
            # ---- constants / one-time setup ----
            iota_t = cpool.tile([P, P], bf16)
            nc.gpsimd.iota(iota_t[:], pattern=[[1, P]], base=0,
                           channel_multiplier=0,
                           allow_small_or_imprecise_dtypes=True)
            ones_t = cpool.tile([P, 1], f32)
            nc.vector.memset(ones_t[:], 1.0)
            ones1_bf = cpool.tile([1, P], bf16)
            nc.vector.memset(ones1_bf[:], 1.0)
            iota_col = cpool.tile([P, 1], f32)
            nc.gpsimd.iota(iota_col[:], pattern=[[1, 1]], base=0,
                           channel_multiplier=1,
                           allow_small_or_imprecise_dtypes=True)
            ident_bf = cpool.tile([P, P], bf16)
            nc.vector.tensor_scalar(
                out=ident_bf[:], in0=iota_t[:], scalar1=iota_col[:],
                scalar2=None, op0=OP.is_equal)

            b1t_t = cpool.tile([P, D], f32)
            nc.sync.dma_start(out=b1t_t[:], in_=b1t_d[:, :])
            b2t_t = cpool.tile([P, D], f32)
            nc.sync.dma_start(out=b2t_t[:], in_=b2t_d[:, :])
            woutt_t = cpool.tile([1, D], f32)
            nc.sync.dma_start(out=woutt_t[:], in_=woutt_d[:, :])
            bout_t = cpool.tile([1, 1], f32)
            nc.sync.dma_start(out=bout_t[:], in_=bout_d[:, :])

            # edge metadata, resident in SBUF
            idx_t = cpool.tile([P, ctot * 8], i16)
            nc.sync.dma_start(out=idx_t[:], in_=idx_d[:, :])
            erel_f = cpool.tile([P, ctot], f32)
            nc.sync.dma_start(out=erel_f[:], in_=erel_d[:, :])

            # W_aug layer 1 (fp32, host-assembled)
            waug1 = cpool.tile([D, DT], f32)
            nc.sync.dma_start(out=waug1[:], in_=waug1_d[:, :])

            # W_aug layer 2: host-assembled fp32, split hi+lo bf16
            waug2f = cpool.tile([D, DT], f32)
            nc.sync.dma_start(out=waug2f[:], in_=waug2_d[:, :])
            waug2_hi = cpool.tile([D, DT], bf16)
            nc.vector.tensor_copy(out=waug2_hi[:], in_=waug2f[:])
            waug2_lo = cpool.tile([D, DT], bf16)
            nc.vector.tensor_tensor(out=waug2_lo[:], in0=waug2f[:],
                                    in1=waug2_hi[:], op=OP.subtract)

            # per-layer per-node SBUF residents (own nodes)
            hstage = cpool.tile([P, cfg.nt, D], bf16)
            sdall = cpool.tile([P, cfg.nt], f32)
            ssall = cpool.tile([P, cfg.nt], f32)
            sdbf = cpool.tile([P, cfg.nt], bf16)
            wself = cpool.tile([P, cfg.nt], f32)

            gsum = psg.tile([1, D], f32, space="PSUM")

            def node_phase(layer):
                hown = h1own if layer == 1 else h2own
                for b0 in range(0, cfg.nt, TBN):
                    b1 = min(b0 + TBN, cfg.nt)
                    nb = b1 - b0
                    if layer == 1:
                        xt_t = sb.tile([D, TBN * P], f32, tag="xt")
                        nc.sync.dma_start(
                            out=xt_t[:, 0:nb * P],
                            in_=xt_d[:, b0 * P:b1 * P])
                    else:
                        xt_t = sb.tile([D, TBN * P], bf16, tag="xt2")
                        nc.sync.dma_start(
                            out=xt_t[:, 0:nb * P],
                            in_=out1t[:, b0 * P:b1 * P])
                    stage_n = sb.tile([P, TBN, DT], bf16, tag="stn")
                    for ti in range(nb):
                        pn = psn.tile([P, DT], f32, space="PSUM")
                        if layer == 1:
                            nc.tensor.matmul(
                                out=pn[:], lhsT=xt_t[:, ti * P:(ti + 1) * P],
                                rhs=waug1[:], start=True, stop=True)
                        else:
                            nc.tensor.matmul(
                                out=pn[:], lhsT=xt_t[:, ti * P:(ti + 1) * P],
                                rhs=waug2_hi[:], start=True, stop=False)
                            nc.tensor.matmul(
                                out=pn[:], lhsT=xt_t[:, ti * P:(ti + 1) * P],
                                rhs=waug2_lo[:], start=False, stop=True)
                        nc.vector.tensor_copy(out=stage_n[:, ti, :], in_=pn[:])
                        nc.vector.memset(stage_n[:, ti, D:D + 1], 1.0)
                    # persist own-node h and scores for self-loop handling
                    nc.vector.tensor_copy(
                        out=hstage[:, b0:b1, :], in_=stage_n[:, 0:nb, 0:D])
                    nc.vector.tensor_copy(
                        out=ssall[:, b0:b1], in_=stage_n[:, 0:nb, D + 1])
                    nc.vector.tensor_copy(
                        out=sdall[:, b0:b1], in_=stage_n[:, 0:nb, D + 2])
                    nc.sync.dma_start(
                        out=hown[b0 * P:b1 * P, :].rearrange(
                            "(k p) d -> p k d", p=P),
                        in_=stage_n[:, 0:nb, :])
                # self-loop weights: w_self = exp(lrelu(s_src + s_dst))
                zs_t = sb.tile([P, cfg.nt], f32, tag="zself")
                nc.vector.tensor_tensor(
                    out=zs_t[:], in0=ssall[:], in1=sdall[:], op=OP.add)
                ls_t = sb.tile([P, cfg.nt], f32, tag="lself")
                nc.vector.scalar_tensor_tensor(
                    out=ls_t[:], in0=zs_t[:], scalar=NEG, in1=zs_t[:],
                    op0=OP.mult, op1=OP.max)
                nc.scalar.activation(out=wself[:], in_=ls_t[:], func=AF.Exp)
                nc.vector.tensor_copy(out=sdbf[:], in_=sdall[:])

            def allgather(layer):
                hown = h1own if layer == 1 else h2own
                tab = tab1 if layer == 1 else tab2
                nc.gpsimd.collective_compute(
                    "AllGather", OP.bypass, replica_groups=rg,
                    ins=[hown[:, :]], outs=[tab[:, :]])

            def gather_batch(layer, b0):
                tab = tab1 if layer == 1 else tab2
                b1 = min(b0 + TBE, cfg.nt)
                nb = b1 - b0
                cs0 = CPT * b0
                gws = []
                c0 = 0
                for g in range(NR):
                    gwid = nb * CAPS[g]
                    nidx = gwid * P
                    g_w = gp.tile([P, TBE * CAPS[g], DT], bf16,
                                  tag=f"g{g}")
                    nc.gpsimd.dma_gather(
                        out_ap=g_w[:, 0:gwid, :],
                        in_ap=tab[cfg.bases[g]:cfg.bases[g] + WIN, :],
                        idxs_ap=idx_t[:, (cs0 + c0) * 8:
                                      (cs0 + c0 + gwid) * 8],
                        num_idxs=nidx, num_idxs_reg=nidx,
                        elem_size=DT, single_packet=False)
                    gws.append(g_w)
                    c0 += gwid
                return gws

            def edge_phase(layer, do_gather=True):
                btile = b1t_t if layer == 1 else b2t_t
                for b0 in range(0, cfg.nt, TBE):
                    b1 = min(b0 + TBE, cfg.nt)
                    nb = b1 - b0
                    cs0 = CPT * b0            # first chunk slot of batch
                    cb = CPT * nb             # chunks in batch
                    # window-group column offsets within the batch
                    goff = [0]
                    for g in range(NR):
                        goff.append(goff[-1] + nb * CAPS[g])

                    def slot_to_gti(c):
                        for g in range(NR):
                            if c < goff[g + 1]:
                                return g, (c - goff[g]) // CAPS[g], \
                                    (c - goff[g]) % CAPS[g]
                        raise AssertionError(c)

                    if do_gather:
                        gws = gather_batch(layer, b0)
                    else:
                        gws = [gp.tile([P, TBE * CAPS[g], DT], bf16,
                                       tag=f"g{g}") for g in range(NR)]

                    erow_t = sb.tile([1, TBE * CPT * P], bf16,
                                     tag="erow", bufs=1)
                    nc.sync.dma_start(
                        out=erow_t[:, 0:cb * P],
                        in_=erelrow_d[cs0:cs0 + cb, :].rearrange(
                            "c p -> (c p)").unsqueeze(0))

                    # s_dst per edge: batched replicate + is_equal + tiny mm
                    psd_b = psd.tile([P, TBE * CPT], f32, space="PSUM")
                    for g0 in range(0, cb, 4):
                        g1 = min(g0 + 4, cb)
                        gsz = g1 - g0
                        prepl = psr.tile([P, 512], f32, space="PSUM")
                        nc.tensor.matmul(
                            out=prepl[:, 0:gsz * P], lhsT=ones1_bf[:],
                            rhs=erow_t[:, g0 * P:g1 * P],
                            start=True, stop=True)
                        ohT = ohp.tile([P, 512], bf16, tag="ohT")
                        nc.vector.tensor_scalar(
                            out=ohT[:, 0:gsz * P], in0=prepl[:, 0:gsz * P],
                            scalar1=iota_col[:], scalar2=None,
                            op0=OP.is_equal)
                        for c in range(g0, g1):
                            gt_tile = b0 + slot_to_gti(c)[1]
                            nc.tensor.matmul(
                                out=psd_b[:, c:c + 1],
                                lhsT=ohT[:, (c - g0) * P:(c - g0 + 1) * P],
                                rhs=sdbf[:, gt_tile:gt_tile + 1],
                                start=True, stop=True, skip_group_check=True)

                    # attention weights per window so each window's
                    # one-hot matmuls unblock on its own gather only
                    z_t = sb.tile([P, TBE * CPT], f32, tag="z")
                    lr_t = sb.tile([P, TBE * CPT], f32, tag="lr")
                    w_t = sb.tile([P, TBE * CPT], f32, tag="w")
                    for g in range(NR):
                        sl = slice(goff[g], goff[g + 1])
                        nc.vector.tensor_tensor(
                            out=z_t[:, sl],
                            in0=gws[g][:, 0:nb * CAPS[g], D + 1],
                            in1=psd_b[:, sl], op=OP.add)
                        nc.vector.scalar_tensor_tensor(
                            out=lr_t[:, sl], in0=z_t[:, sl], scalar=NEG,
                            in1=z_t[:, sl], op0=OP.mult, op1=OP.max)
                        nc.scalar.activation(out=w_t[:, sl], in_=lr_t[:, sl],
                                             func=AF.Exp)

                    if layer == 1:
                        stage_xt = sb.tile([D, TBE * P], bf16, tag="sxt")
                    for ti in range(nb):
                        gt = b0 + ti
                        pe_t = pse.tile([P, D + 1], f32, space="PSUM")
                        j = 0
                        for g in range(NR):
                            for cc in range(CAPS[g]):
                                c = goff[g] + ti * CAPS[g] + cc
                                oh = ohp.tile([P, P], bf16, tag="oh")
                                nc.vector.tensor_scalar(
                                    out=oh[:], in0=iota_t[:],
                                    scalar1=erel_f[:, cs0 + c:cs0 + c + 1],
                                    scalar2=w_t[:, c:c + 1],
                                    op0=OP.is_equal, op1=OP.mult)
                                nc.tensor.matmul(
                                    out=pe_t[:], lhsT=oh[:],
                                    rhs=gws[g][:, ti * CAPS[g] + cc, 0:D + 1],
                                    start=(j == 0), stop=(j == CPT - 1))
                                j += 1
                        # finalize tile: add self-loop, normalize, bias, relu
                        rows = cfg.last_rows if gt == cfg.nt - 1 else P
                        den = sb.tile([P, 1], f32, tag="den")
                        nc.vector.tensor_scalar(
                            out=den[:], in0=pe_t[:, D:D + 1],
                            scalar1=wself[:, gt:gt + 1], scalar2=None,
                            op0=OP.add)
                        recip = sb.tile([P, 1], f32, tag="rc")
                        nc.vector.reciprocal(out=recip[:], in_=den[:])
                        num = sb.tile([P, D], f32, tag="num")
                        nc.vector.scalar_tensor_tensor(
                            out=num[:], in0=hstage[:, gt, :],
                            scalar=wself[:, gt:gt + 1], in1=pe_t[:, 0:D],
                            op0=OP.mult, op1=OP.add)
                        o2 = sb.tile([P, D], f32, tag="o2")
                        nc.vector.scalar_tensor_tensor(
                            out=o2[:], in0=num[:], scalar=recip[:],
                            in1=btile[:], op0=OP.mult, op1=OP.add)
                        if layer == 1:
                            o3 = sb.tile([P, D], bf16, tag="o3")
                            nc.scalar.activation(out=o3[:], in_=o2[:],
                                                 func=AF.Relu)
                            pt = pst.tile([D, P], bf16, space="PSUM",
                                          tag="ptr")
                            nc.tensor.transpose(pt[:], o3[:], ident_bf[:])
                            nc.vector.tensor_copy(
                                out=stage_xt[:, ti * P:(ti + 1) * P],
                                in_=pt[:])
                        else:
                            o3 = sb.tile([P, D], f32, tag="o3f")
                            nc.scalar.activation(out=o3[:], in_=o2[:],
                                                 func=AF.Relu)
                            nc.tensor.matmul(
                                out=gsum[:], lhsT=ones_t[0:rows, :],
                                rhs=o3[0:rows, :],
                                start=(gt == 0), stop=(gt == cfg.nt - 1),
                                skip_group_check=True)
                    if layer == 1:
                        nc.sync.dma_start(
                            out=out1t[:, b0 * P:b1 * P],
                            in_=stage_xt[:, 0:nb * P])

            # ---- drive phases (rep/rep_mode are measurement hooks) ----
            for layer in (1, 2):
                node_phase(layer)
                allgather(layer)
                edge_phase(layer)
            for _ in range(rep - 1):
                if rep_mode == "all":
                    for layer in (1, 2):
                        node_phase(layer)
                        allgather(layer)
                        edge_phase(layer)
                elif rep_mode == "coll":
                    for layer in (1, 2):
                        allgather(layer)
                elif rep_mode == "gather":
                    for layer in (1, 2):
                        for b0 in range(0, cfg.nt, TBE):
                            gather_batch(layer, b0)
                elif rep_mode == "gatherhalf":
                    # windows 0..2 only (7/12 of the descriptors)
                    for layer in (1, 2):
                        for b0 in range(0, cfg.nt, TBE):
                            b1 = min(b0 + TBE, cfg.nt)
                            nb = b1 - b0
                            cs0 = CPT * b0
                            tab = tab1 if layer == 1 else tab2
                            c0 = 0
                            for g in range(3):
                                gwid = nb * CAPS[g]
                                nidx = gwid * P
                                g_w = gp.tile([P, TBE * CAPS[g], DT], bf16,
                                              tag=f"g{g}")
                                nc.gpsimd.dma_gather(
                                    out_ap=g_w[:, 0:gwid, :],
                                    in_ap=tab[cfg.bases[g]:cfg.bases[g] + WIN, :],
                                    idxs_ap=idx_t[:, (cs0 + c0) * 8:
                                                  (cs0 + c0 + gwid) * 8],
                                    num_idxs=nidx, num_idxs_reg=nidx,
                                    elem_size=DT, single_packet=False)
                                c0 += gwid
                elif rep_mode == "edge":
                    for layer in (1, 2):
                        edge_phase(layer)
                elif rep_mode == "edgecompute":
                    for layer in (1, 2):
                        edge_phase(layer, do_gather=False)
                elif rep_mode == "node":
                    for layer in (1, 2):
                        node_phase(layer)
                else:
                    raise ValueError(rep_mode)

            # ---- head: mean pool + linear ----
            g_sb = sb.tile([1, D], f32, tag="gsb")
            nc.vector.tensor_copy(out=g_sb[:], in_=gsum[:])
            nc.sync.dma_start(out=gin[:, :], in_=g_sb[:])
            nc.gpsimd.collective_compute(
                "AllReduce", OP.add, replica_groups=rg,
                ins=[gin[:, :]], outs=[gout[:, :]])
            g2_sb = sb.tile([1, D], f32, tag="g2sb")
            nc.sync.dma_start(out=g2_sb[:], in_=gout[:, :])
            junk = sb.tile([1, D], f32, tag="junk")
            res = sb.tile([1, 1], f32, tag="res")
            nc.vector.scalar_tensor_tensor(
                out=junk[:], in0=g2_sb[:], scalar=1.0 / cfg.n_nodes,
                in1=woutt_t[:], op0=OP.mult, op1=OP.mult,
                accum_out=res[:])
            res2 = sb.tile([1, 1], f32, tag="res2")
            nc.vector.tensor_tensor(out=res2[:], in0=res[:], in1=bout_t[:],
                                    op=OP.add)
            nc.sync.dma_start(out=out_d[:, :], in_=res2[:])

            if debug_dumps:
                for nm, src, shp, dt_ in [
                    ("d_h1", h1own, [cfg.npad, DT], bf16),
                    ("d_tab1", tab1, [cfg.ntab, DT], bf16),
                    ("d_out1t", out1t, [D, cfg.npad], bf16),
                    ("d_h2", h2own, [cfg.npad, DT], bf16),
                    ("d_gin", gin, [1, D], f32),
                    ("d_gout", gout, [1, D], f32),
                ]:
                    dd = nc.dram_tensor(nm, shp, dt_, kind="ExternalOutput")
                    nc.sync.dma_start(out=dd[:, :], in_=src[:, :])

    nc.compile()
    return nc


def kernel(**inputs):
    cfg = Cfg(N_NODES)
    in_maps, c_fix = prep_inputs(inputs, cfg)
    nc = build_kernel(cfg, c_fix)
    from concourse.bass_utils import run_bass_kernel_spmd
    res = run_bass_kernel_spmd(nc, in_maps, list(range(CORES)))
    return np.asarray(res.results[0]["out"], np.float32)
